# revision 3
# baseline (speedup 1.0000x reference)
"""Bahdanau attention Trainium2 kernel.

Full (unsharded) inputs in, full outputs out. Internally: data-parallel over
batch (B=32 -> 4 per core x 8 cores), weights replicated, no collectives.

Per core, per batch element:
  v_proj = values @ W2 + (query @ W1 + b1 + b2)   [T, U]   (PE, fp32r: 1 cyc/row)
  z      = tanh(v_proj)                            [T, U]   (ScalarE)
  score  = z @ V_w                                 [T]      (VectorE fused mul+reduce)
  attn   = softmax(score)                          [T]      (no max-subtraction:
                                                             |score| <= ||V_w||_1, safe)
  context= attn @ values                           [D]      (PE, exp-weighted, scaled
                                                             by 1/sum at the end)

values is loaded once per batch with a casting DMA (fp32 -> fp32r) so both the
PE transposes (for the D-contraction) and the context matmul run in fp32r at
full PE rate. fp32r rounding error measured ~1e-4 on HW.
"""

import numpy as np

B, T, D, U = 32, 2048, 512, 512
N_CORES = 8
BPC = B // N_CORES  # batches per core
KD = D // 128       # k-tiles over D
NT = T // 128       # t-tiles over T

_CACHE = {}


def _build_program():
    from contextlib import ExitStack

    import concourse.bacc as bacc
    import concourse.tile as tile
    from concourse import mybir
    from concourse.masks import make_identity

    f32 = mybir.dt.float32
    f32r = mybir.dt.float32r

    nc = bacc.Bacc("TRN2", target_bir_lowering=False, debug=False,
                   num_devices=N_CORES)

    query = nc.dram_tensor("query", [BPC, D], f32, kind="ExternalInput").ap()
    values = nc.dram_tensor("values", [BPC, T, D], f32, kind="ExternalInput").ap()
    W1 = nc.dram_tensor("W1_w", [D, U], f32, kind="ExternalInput").ap()
    W2 = nc.dram_tensor("W2_w", [D, U], f32, kind="ExternalInput").ap()
    Vw = nc.dram_tensor("V_w", [U, 1], f32, kind="ExternalInput").ap()
    qb = nc.dram_tensor("qb", [U], f32, kind="ExternalInput").ap()
    ctx_out = nc.dram_tensor("context", [BPC, D], f32, kind="ExternalOutput").ap()
    attn_out = nc.dram_tensor("attn", [BPC, T], f32, kind="ExternalOutput").ap()

    with tile.TileContext(nc) as tc, ExitStack() as ctx:
        singles = ctx.enter_context(tc.tile_pool(name="singles", bufs=1))
        vnat_pool = ctx.enter_context(tc.tile_pool(name="vnat", bufs=2))
        vt_pool = ctx.enter_context(tc.tile_pool(name="vt", bufs=2))
        z_pool = ctx.enter_context(tc.tile_pool(name="z", bufs=3))
        small = ctx.enter_context(tc.tile_pool(name="small", bufs=2))
        tp_psum = ctx.enter_context(tc.tile_pool(name="tp_psum", bufs=2, space="PSUM"))
        vp_psum = ctx.enter_context(tc.tile_pool(name="vp_psum", bufs=2, space="PSUM"))
        misc_psum = ctx.enter_context(tc.tile_pool(name="misc_psum", bufs=1, space="PSUM"))

        # ---- constants / weights ----
        ident_f = singles.tile([128, 128], f32)
        make_identity(nc, ident_f)
        ident_r = singles.tile([128, 128], f32r)
        nc.vector.tensor_copy(out=ident_r, in_=ident_f)

        ones_row = singles.tile([1, 128], f32)
        nc.vector.memset(ones_row, 1.0)
        ones_row_r = singles.tile([1, 128], f32r)
        nc.vector.tensor_copy(out=ones_row_r, in_=ones_row)
        ones_col = singles.tile([128, 1], f32)
        nc.vector.memset(ones_col, 1.0)

        w1_sb = singles.tile([128, KD, U], f32)
        nc.sync.dma_start(out=w1_sb, in_=W1.rearrange("(k p) u -> p k u", p=128))
        w2_sb = singles.tile([128, KD, U], f32r)
        nc.gpsimd.dma_start(out=w2_sb, in_=W2.rearrange("(k p) u -> p k u", p=128))
        v_bcast = singles.tile([128, U], f32)
        nc.gpsimd.dma_start(
            out=v_bcast, in_=Vw.rearrange("u one -> one u").to_broadcast([128, U])
        )
        qb_sb = singles.tile([1, U], f32)
        nc.sync.dma_start(out=qb_sb, in_=qb[None, :])

        # ---- q_proj = query @ W1 + qb (fp32, exact), rows -> partition 0 ----
        q_sb = singles.tile([BPC, D], f32)
        nc.sync.dma_start(out=q_sb, in_=query)
        qT = singles.tile([128, KD, BPC], f32)
        for kd in range(KD):
            tp = tp_psum.tile([128, BPC], f32, tag="tp")
            nc.tensor.transpose(tp, q_sb[:, kd * 128:(kd + 1) * 128],
                                ident_f[0:BPC, 0:BPC])
            nc.scalar.copy(out=qT[:, kd, :], in_=tp)
        qp_ps = vp_psum.tile([BPC, U], f32, tag="vp")
        for kd in range(KD):
            nc.tensor.matmul(qp_ps, lhsT=qT[:, kd, :], rhs=w1_sb[:, kd, :],
                             start=(kd == 0), stop=False)
        nc.tensor.matmul(qp_ps, lhsT=ones_row[:, 0:BPC], rhs=qb_sb,
                         start=False, stop=True)
        qp_sb = singles.tile([BPC, U], f32)
        nc.scalar.copy(out=qp_sb, in_=qp_ps)
        qpb_rows = singles.tile([1, BPC, U], f32)
        for b in range(BPC):
            nc.gpsimd.dma_start(out=qpb_rows[:, b, :], in_=qp_sb[b:b + 1, :])
        qpb_rows_r = singles.tile([1, BPC, U], f32r)
        nc.vector.tensor_copy(out=qpb_rows_r, in_=qpb_rows)

        # ---- main loop over batch elements ----
        for b in range(BPC):
            # load values[b] as fp32r (casting DMA): [128(t), NT, D]
            vnat = vnat_pool.tile([128, NT, D], f32r, tag="vnat")
            vsrc = values[b].rearrange("(n p) d -> p n d", p=128)
            for i in range(4):
                nc.gpsimd.dma_start(out=vnat[:, 4 * i:4 * i + 4, :],
                                    in_=vsrc[:, 4 * i:4 * i + 4, :])

            # transpose to [128(d), KD, T] fp32r
            vt = vt_pool.tile([128, KD, T], f32r, tag="vt")
            for n in range(NT):
                tp = tp_psum.tile([128, KD * 128], f32r, tag="tp")
                for kd in range(KD):
                    nc.tensor.transpose(
                        tp[:, kd * 128:(kd + 1) * 128],
                        vnat[:, n, kd * 128:(kd + 1) * 128],
                        ident_r,
                    )
                dst = vt[:, :, n * 128:(n + 1) * 128]
                src = tp.rearrange("p (k t) -> p k t", k=KD)
                if n % 2 == 0:
                    nc.scalar.copy(out=dst, in_=src)
                else:
                    nc.vector.tensor_copy(out=dst, in_=src)

            # v_proj + bias, tanh, score
            score_sb = small.tile([128, NT], f32, tag="score")
            for n in range(NT):
                vp = vp_psum.tile([128, U], f32, tag="vp")
                nc.tensor.matmul(vp, lhsT=ones_row_r,
                                 rhs=qpb_rows_r[:, b, :],
                                 start=True, stop=False)
                for kd in range(KD):
                    nc.tensor.matmul(
                        vp,
                        lhsT=vt[:, kd, n * 128:(n + 1) * 128],
                        rhs=w2_sb[:, kd, :],
                        start=False, stop=(kd == KD - 1),
                    )
                z = z_pool.tile([128, U], f32, tag="z")
                nc.scalar.activation(out=z, in_=vp,
                                     func=mybir.ActivationFunctionType.Tanh)
                zv = z_pool.tile([128, U], f32, tag="zv")
                nc.vector.scalar_tensor_tensor(
                    out=zv, in0=z, scalar=1.0, in1=v_bcast,
                    op0=mybir.AluOpType.mult, op1=mybir.AluOpType.mult,
                    accum_out=score_sb[:, n:n + 1],
                )

            # softmax over all T (no max-subtraction; scores are bounded)
            exp_sb = small.tile([128, NT], f32r, tag="exp")
            nc.scalar.activation(out=exp_sb, in_=score_sb,
                                 func=mybir.ActivationFunctionType.Exp)
            sum_col = small.tile([128, 1], f32, tag="sumcol")
            nc.vector.reduce_sum(sum_col, exp_sb, axis=mybir.AxisListType.X)
            tot_ps = misc_psum.tile([1, 1], f32, tag="tot")
            nc.tensor.matmul(tot_ps, lhsT=sum_col, rhs=ones_col,
                             start=True, stop=True)
            tot_sb = small.tile([1, 1], f32, tag="tot_sb")
            nc.scalar.copy(out=tot_sb, in_=tot_ps)
            rb_ps = misc_psum.tile([128, 1], f32, tag="rb")
            nc.tensor.matmul(rb_ps, lhsT=ones_row, rhs=tot_sb,
                             start=True, stop=True)
            recip_sb = small.tile([128, 1], f32, tag="recip")
            nc.vector.reciprocal(recip_sb, rb_ps)

            # attn output
            attn_sb = small.tile([128, NT], f32, tag="attn")
            nc.vector.tensor_scalar_mul(attn_sb, exp_sb, recip_sb)
            nc.sync.dma_start(out=attn_out[b].rearrange("(n p) -> p n", p=128),
                              in_=attn_sb)

            # context = (exp . values) / tot
            ctx_ps = misc_psum.tile([1, D], f32, tag="ctx")
            for n in range(NT):
                nc.tensor.matmul(ctx_ps, lhsT=exp_sb[:, n:n + 1],
                                 rhs=vnat[:, n, :],
                                 start=(n == 0), stop=(n == NT - 1))
            ctx_sb = small.tile([1, D], f32, tag="ctx_sb")
            nc.scalar.activation(out=ctx_sb, in_=ctx_ps,
                                 func=mybir.ActivationFunctionType.Copy,
                                 bias=0.0, scale=recip_sb[0:1, 0:1])
            nc.sync.dma_start(out=ctx_out[b][None, :], in_=ctx_sb)

    nc.compile()
    return nc


def _get_program():
    if "nc" not in _CACHE:
        _CACHE["nc"] = _build_program()
    return _CACHE["nc"]


def _make_in_maps(query, values, W1_w, W1_b, W2_w, W2_b, V_w, V_b):
    query = np.ascontiguousarray(np.asarray(query, dtype=np.float32))
    values = np.ascontiguousarray(np.asarray(values, dtype=np.float32))
    W1_w = np.ascontiguousarray(np.asarray(W1_w, dtype=np.float32))
    W2_w = np.ascontiguousarray(np.asarray(W2_w, dtype=np.float32))
    V_w = np.ascontiguousarray(np.asarray(V_w, dtype=np.float32))
    qb = (np.asarray(W1_b, dtype=np.float32)
          + np.asarray(W2_b, dtype=np.float32)).astype(np.float32)
    # V_b drops out of softmax (shift invariance) -> not an input to the kernel
    in_maps = []
    for c in range(N_CORES):
        sl = slice(c * BPC, (c + 1) * BPC)
        in_maps.append({
            "query": np.ascontiguousarray(query[sl]),
            "values": np.ascontiguousarray(values[sl]),
            "W1_w": W1_w,
            "W2_w": W2_w,
            "V_w": V_w,
            "qb": qb,
        })
    return in_maps


def kernel(query, values, W1_w, W1_b, W2_w, W2_b, V_w, V_b):
    from concourse.bass_utils import run_bass_kernel_spmd

    nc = _get_program()
    in_maps = _make_in_maps(query, values, W1_w, W1_b, W2_w, W2_b, V_w, V_b)
    res = run_bass_kernel_spmd(nc, in_maps, list(range(N_CORES)))
    context = np.concatenate([res.results[c]["context"] for c in range(N_CORES)], axis=0)
    attn = np.concatenate([res.results[c]["attn"] for c in range(N_CORES)], axis=0)
    return context.astype(np.float32), attn[..., None].astype(np.float32)


# revision 5
# speedup vs baseline: 3.5652x; 3.5652x over previous
"""Bahdanau attention Trainium2 kernel.

Full (unsharded) inputs in, full outputs out. Internally: data-parallel over
batch (B=32 -> 4 per core x 8 cores), weights replicated, no collectives.

Per core, per batch element:
  v_proj = values @ W2 + (query @ W1 + b1 + b2)   [T, U]   (PE, fp32r: 1 cyc/row)
  z      = tanh(v_proj)                            [T, U]   (ScalarE)
  score  = z @ V_w                                 [T]      (VectorE fused mul+reduce)
  attn   = softmax(score)                          [T]      (no max-subtraction:
                                                             |score| <= ||V_w||_1, safe)
  context= attn @ values                           [D]      (PE, exp-weighted, scaled
                                                             by 1/sum at the end)

values is loaded once per batch with a casting DMA (fp32 -> fp32r) so both the
PE transposes (for the D-contraction) and the context matmul run in fp32r at
full PE rate. fp32r rounding error measured ~1e-4 on HW.
"""

import numpy as np

B, T, D, U = 32, 2048, 512, 512
N_CORES = 8
BPC = B // N_CORES  # batches per core
KD = D // 128       # k-tiles over D
NT = T // 128       # t-tiles over T

_CACHE = {}


def _build_program(repeat=1):
    from contextlib import ExitStack

    import concourse.bacc as bacc
    import concourse.tile as tile
    from concourse import mybir
    from concourse.masks import make_identity

    f32 = mybir.dt.float32
    f32r = mybir.dt.float32r

    nc = bacc.Bacc("TRN2", target_bir_lowering=False, debug=False,
                   num_devices=N_CORES)

    query = nc.dram_tensor("query", [BPC, D], f32, kind="ExternalInput").ap()
    values = nc.dram_tensor("values", [BPC, T, D], f32, kind="ExternalInput").ap()
    W1 = nc.dram_tensor("W1_w", [D, U], f32, kind="ExternalInput").ap()
    W2 = nc.dram_tensor("W2_w", [D, U], f32, kind="ExternalInput").ap()
    Vw = nc.dram_tensor("V_w", [U, 1], f32, kind="ExternalInput").ap()
    qb = nc.dram_tensor("qb", [U], f32, kind="ExternalInput").ap()
    ctx_out = nc.dram_tensor("context", [BPC, D], f32, kind="ExternalOutput").ap()
    attn_out = nc.dram_tensor("attn", [BPC, T], f32, kind="ExternalOutput").ap()

    with tile.TileContext(nc) as tc, ExitStack() as ctx:
        singles = ctx.enter_context(tc.tile_pool(name="singles", bufs=1))
        vnat_pool = ctx.enter_context(tc.tile_pool(name="vnat", bufs=2))
        vt_pool = ctx.enter_context(tc.tile_pool(name="vt", bufs=2))
        z_pool = ctx.enter_context(tc.tile_pool(name="z", bufs=3))
        small = ctx.enter_context(tc.tile_pool(name="small", bufs=2))
        tp_psum = ctx.enter_context(tc.tile_pool(name="tp_psum", bufs=2, space="PSUM"))
        vp_psum = ctx.enter_context(tc.tile_pool(name="vp_psum", bufs=2, space="PSUM"))
        misc_psum = ctx.enter_context(tc.tile_pool(name="misc_psum", bufs=1, space="PSUM"))

        # ---- constants / weights ----
        ident_f = singles.tile([128, 128], f32)
        make_identity(nc, ident_f)
        ident_r = singles.tile([128, 128], f32r)
        nc.vector.tensor_copy(out=ident_r, in_=ident_f)

        ones_row = singles.tile([1, 128], f32)
        nc.vector.memset(ones_row, 1.0)
        ones_row_r = singles.tile([1, 128], f32r)
        nc.vector.tensor_copy(out=ones_row_r, in_=ones_row)
        ones_col = singles.tile([128, 1], f32)
        nc.vector.memset(ones_col, 1.0)

        w1_sb = singles.tile([128, KD, U], f32)
        nc.sync.dma_start(out=w1_sb, in_=W1.rearrange("(k p) u -> p k u", p=128))
        w2_sb = singles.tile([128, KD, U], f32r)
        nc.gpsimd.dma_start(out=w2_sb, in_=W2.rearrange("(k p) u -> p k u", p=128))
        v_bcast = singles.tile([128, U], f32)
        nc.gpsimd.dma_start(
            out=v_bcast, in_=Vw.rearrange("u one -> one u").to_broadcast([128, U])
        )
        qb_sb = singles.tile([1, U], f32)
        nc.sync.dma_start(out=qb_sb, in_=qb[None, :])

        # ---- q_proj = query @ W1 + qb (fp32, exact), rows -> partition 0 ----
        q_sb = singles.tile([BPC, D], f32)
        nc.sync.dma_start(out=q_sb, in_=query)
        qT = singles.tile([128, KD, BPC], f32)
        for kd in range(KD):
            tp = tp_psum.tile([128, BPC], f32, tag="tp")
            nc.tensor.transpose(tp, q_sb[:, kd * 128:(kd + 1) * 128],
                                ident_f[0:BPC, 0:BPC])
            nc.scalar.copy(out=qT[:, kd, :], in_=tp)
        qp_ps = vp_psum.tile([BPC, U], f32, tag="vp")
        for kd in range(KD):
            nc.tensor.matmul(qp_ps, lhsT=qT[:, kd, :], rhs=w1_sb[:, kd, :],
                             start=(kd == 0), stop=False)
        nc.tensor.matmul(qp_ps, lhsT=ones_row[:, 0:BPC], rhs=qb_sb,
                         start=False, stop=True)
        qp_sb = singles.tile([BPC, U], f32)
        nc.scalar.copy(out=qp_sb, in_=qp_ps)
        qpb_rows = singles.tile([1, BPC, U], f32)
        for b in range(BPC):
            nc.gpsimd.dma_start(out=qpb_rows[:, b, :], in_=qp_sb[b:b + 1, :])
        qpb_rows_r = singles.tile([1, BPC, U], f32r)
        nc.vector.tensor_copy(out=qpb_rows_r, in_=qpb_rows)

        # ---- main loop over batch elements ----
        for b in [b for _ in range(repeat) for b in range(BPC)]:
            # load values[b] as fp32r (casting DMA): [128(t), NT, D]
            vnat = vnat_pool.tile([128, NT, D], f32r, tag="vnat")
            vsrc = values[b].rearrange("(n p) d -> p n d", p=128)
            for i in range(4):
                nc.gpsimd.dma_start(out=vnat[:, 4 * i:4 * i + 4, :],
                                    in_=vsrc[:, 4 * i:4 * i + 4, :])

            # transpose to [128(d), KD, T] fp32r
            vt = vt_pool.tile([128, KD, T], f32r, tag="vt")
            for n in range(NT):
                tp = tp_psum.tile([128, KD * 128], f32r, tag="tp")
                for kd in range(KD):
                    nc.tensor.transpose(
                        tp[:, kd * 128:(kd + 1) * 128],
                        vnat[:, n, kd * 128:(kd + 1) * 128],
                        ident_r,
                    )
                dst = vt[:, :, n * 128:(n + 1) * 128]
                src = tp.rearrange("p (k t) -> p k t", k=KD)
                if n % 2 == 0:
                    nc.scalar.copy(out=dst, in_=src)
                else:
                    nc.vector.tensor_copy(out=dst, in_=src)

            # v_proj + bias, tanh, score
            score_sb = small.tile([128, NT], f32, tag="score")
            for n in range(NT):
                vp = vp_psum.tile([128, U], f32, tag="vp")
                nc.tensor.matmul(vp, lhsT=ones_row_r,
                                 rhs=qpb_rows_r[:, b, :],
                                 start=True, stop=False)
                for kd in range(KD):
                    nc.tensor.matmul(
                        vp,
                        lhsT=vt[:, kd, n * 128:(n + 1) * 128],
                        rhs=w2_sb[:, kd, :],
                        start=False, stop=(kd == KD - 1),
                    )
                z = z_pool.tile([128, U], f32, tag="z")
                nc.scalar.activation(out=z, in_=vp,
                                     func=mybir.ActivationFunctionType.Tanh)
                zv = z_pool.tile([128, U], f32, tag="zv")
                nc.vector.scalar_tensor_tensor(
                    out=zv, in0=z, scalar=1.0, in1=v_bcast,
                    op0=mybir.AluOpType.mult, op1=mybir.AluOpType.mult,
                    accum_out=score_sb[:, n:n + 1],
                )

            # softmax over all T (no max-subtraction; scores are bounded)
            exp_sb = small.tile([128, NT], f32r, tag="exp")
            nc.scalar.activation(out=exp_sb, in_=score_sb,
                                 func=mybir.ActivationFunctionType.Exp)
            sum_col = small.tile([128, 1], f32, tag="sumcol")
            nc.vector.reduce_sum(sum_col, exp_sb, axis=mybir.AxisListType.X)
            tot_ps = misc_psum.tile([1, 1], f32, tag="tot")
            nc.tensor.matmul(tot_ps, lhsT=sum_col, rhs=ones_col,
                             start=True, stop=True)
            tot_sb = small.tile([1, 1], f32, tag="tot_sb")
            nc.scalar.copy(out=tot_sb, in_=tot_ps)
            rb_ps = misc_psum.tile([128, 1], f32, tag="rb")
            nc.tensor.matmul(rb_ps, lhsT=ones_row, rhs=tot_sb,
                             start=True, stop=True)
            recip_sb = small.tile([128, 1], f32, tag="recip")
            nc.vector.reciprocal(recip_sb, rb_ps)

            # attn output
            attn_sb = small.tile([128, NT], f32, tag="attn")
            nc.vector.tensor_scalar_mul(attn_sb, exp_sb, recip_sb)
            nc.sync.dma_start(out=attn_out[b].rearrange("(n p) -> p n", p=128),
                              in_=attn_sb)

            # context = (exp . values) / tot
            ctx_ps = misc_psum.tile([1, D], f32, tag="ctx")
            for n in range(NT):
                nc.tensor.matmul(ctx_ps, lhsT=exp_sb[:, n:n + 1],
                                 rhs=vnat[:, n, :],
                                 start=(n == 0), stop=(n == NT - 1))
            ctx_sb = small.tile([1, D], f32, tag="ctx_sb")
            nc.scalar.activation(out=ctx_sb, in_=ctx_ps,
                                 func=mybir.ActivationFunctionType.Copy,
                                 bias=0.0, scale=recip_sb[0:1, 0:1])
            nc.sync.dma_start(out=ctx_out[b][None, :], in_=ctx_sb)

    nc.compile()
    return nc


def _get_program():
    if "nc" not in _CACHE:
        _CACHE["nc"] = _build_program()
    return _CACHE["nc"]


def _make_in_maps(query, values, W1_w, W1_b, W2_w, W2_b, V_w, V_b):
    query = np.ascontiguousarray(np.asarray(query, dtype=np.float32))
    values = np.ascontiguousarray(np.asarray(values, dtype=np.float32))
    W1_w = np.ascontiguousarray(np.asarray(W1_w, dtype=np.float32))
    W2_w = np.ascontiguousarray(np.asarray(W2_w, dtype=np.float32))
    V_w = np.ascontiguousarray(np.asarray(V_w, dtype=np.float32))
    qb = (np.asarray(W1_b, dtype=np.float32)
          + np.asarray(W2_b, dtype=np.float32)).astype(np.float32)
    # V_b drops out of softmax (shift invariance) -> not an input to the kernel
    in_maps = []
    for c in range(N_CORES):
        sl = slice(c * BPC, (c + 1) * BPC)
        in_maps.append({
            "query": np.ascontiguousarray(query[sl]),
            "values": np.ascontiguousarray(values[sl]),
            "W1_w": W1_w,
            "W2_w": W2_w,
            "V_w": V_w,
            "qb": qb,
        })
    return in_maps


def kernel(query, values, W1_w, W1_b, W2_w, W2_b, V_w, V_b):
    from concourse.bass_utils import run_bass_kernel_spmd

    nc = _get_program()
    in_maps = _make_in_maps(query, values, W1_w, W1_b, W2_w, W2_b, V_w, V_b)
    res = run_bass_kernel_spmd(nc, in_maps, list(range(N_CORES)))
    context = np.concatenate([res.results[c]["context"] for c in range(N_CORES)], axis=0)
    attn = np.concatenate([res.results[c]["attn"] for c in range(N_CORES)], axis=0)
    return context.astype(np.float32), attn[..., None].astype(np.float32)


# revision 21
# speedup vs baseline: 6.3615x; 1.7843x over previous
"""Bahdanau attention Trainium2 kernel.

Full (unsharded) inputs in, full outputs out. Internally: data-parallel over
batch (B=32 -> 4 per core x 8 cores), weights replicated, no collectives.

Per core, per batch element:
  v_projT = W2.T @ values.T + qpb  [U, T]  (PE fp16 operands / fp32 accum,
                                            W2 chunks stationary)
  z       = tanh(v_projT)          [U, T]  (ScalarE, bias per-partition)
  score   = V_w.T @ z              [1, T]  (PE, V stationary)
  attn    = softmax(score)         [T]     (fp32; no max-subtraction:
                                            |score| <= ||V_w||_1, bounded)
  context = attn @ values          [D]     (PE, exp-weighted, 1/sum at the end)

All matmul operands are fp16 (1 cycle/column moving-operand rate; fp32 and
fp32r stream at 4 and 2 cycles/column). fp16's 11-bit mantissa on this data
(|values| < 6, |z| <= 1, weights ~0.02) keeps rel err ~1e-4; accumulation is
always fp32 in PSUM and softmax math is fp32.

T is tiled p-major (partition p holds t = p*16+n) so the values load is 128
contiguous 32KB reads per batch through the casting DMA (fp32 -> fp16), and
the attn store is contiguous per partition. values.T is produced on-chip with
PE transpose-mode matmuls (fp16: 1 cyc/row).
"""

import numpy as np

B, T, D, U = 32, 2048, 512, 512
N_CORES = 8
BPC = B // N_CORES  # batches per core
KD = D // 128       # k-tiles over D
KU = U // 128       # tiles over U
NT = T // 128       # t-tiles over T
TC = T // 512       # 512-wide t-chunks

_CACHE = {}


def _build_program(repeat=1):
    from contextlib import ExitStack

    import concourse.bacc as bacc
    import concourse.tile as tile
    from concourse import mybir
    from concourse.masks import make_identity

    f32 = mybir.dt.float32
    f16 = mybir.dt.float16

    nc = bacc.Bacc("TRN2", target_bir_lowering=False, debug=False,
                   num_devices=N_CORES)

    query = nc.dram_tensor("query", [BPC, D], f32, kind="ExternalInput").ap()
    values = nc.dram_tensor("values", [BPC, T, D], f32, kind="ExternalInput").ap()
    W1 = nc.dram_tensor("W1_w", [D, U], f32, kind="ExternalInput").ap()
    W2 = nc.dram_tensor("W2_w", [D, U], f32, kind="ExternalInput").ap()
    Vw = nc.dram_tensor("V_w", [U, 1], f32, kind="ExternalInput").ap()
    qb = nc.dram_tensor("qb", [U], f32, kind="ExternalInput").ap()
    ctx_out = nc.dram_tensor("context", [BPC, D], f32, kind="ExternalOutput").ap()
    attn_out = nc.dram_tensor("attn", [BPC, T], f32, kind="ExternalOutput").ap()
    score_dram = nc.dram_tensor("score_scratch", [BPC, T], f32)

    with tile.TileContext(nc) as tc, ExitStack() as ctx:
        singles = ctx.enter_context(tc.tile_pool(name="singles", bufs=1))
        vnat_pool = ctx.enter_context(tc.tile_pool(name="vnat", bufs=3))
        NQ = NT // 2

        def load_vnat(b):
            # values[b] as fp16, p-major: half q holds n in [q*8, q*8+8)
            # -> per partition 16KB contiguous DRAM reads (casting DMA)
            vsrc = values[b].rearrange("(p n) d -> p n d", p=128)
            qs = []
            for q in range(2):
                t = vnat_pool.tile([128, NQ, D], f16, tag=f"vnat{q}")
                nc.gpsimd.dma_start(out=t, in_=vsrc[:, q * NQ:(q + 1) * NQ, :])
                qs.append(t)
            return qs
        vt_pool = ctx.enter_context(tc.tile_pool(name="vt", bufs=3))
        stage_pool = ctx.enter_context(tc.tile_pool(name="stage", bufs=1))
        z_pool = ctx.enter_context(tc.tile_pool(name="z", bufs=3))
        small = ctx.enter_context(tc.tile_pool(name="small", bufs=2))
        tp_psum = ctx.enter_context(tc.tile_pool(name="tp_psum", bufs=3, space="PSUM"))
        vp_psum = ctx.enter_context(tc.tile_pool(name="vp_psum", bufs=3, space="PSUM"))
        sc_psum = ctx.enter_context(tc.tile_pool(name="sc_psum", bufs=2, space="PSUM"))

        # ---- identity first (gpsimd), then prefetch batch 0 ----
        ident_f = singles.tile([128, 128], f32)
        make_identity(nc, ident_f)
        ident_h = singles.tile([128, 128], f16)
        nc.vector.tensor_copy(out=ident_h, in_=ident_f)

        batch_list = [b for _ in range(repeat) for b in range(BPC)]
        ones_row = singles.tile([1, 128], f32)
        nc.vector.memset(ones_row, 1.0)
        ones_col = singles.tile([128, 1], f32)
        nc.vector.memset(ones_col, 1.0)

        w1_sb = singles.tile([128, KD, U], f32)
        nc.sync.dma_start(out=w1_sb, in_=W1.rearrange("(k p) u -> p k u", p=128))
        # W2 as fp16 [d-part, kd, u]; stationary chunk = w2[:, kd, u0:u0+128]
        w2_sb = singles.tile([128, KD, U], f16)
        nc.gpsimd.dma_start(out=w2_sb, in_=W2.rearrange("(k p) u -> p k u", p=128))
        # V as fp16 [u-part, ku] for score lhsT chunks
        vT_sb = singles.tile([128, KU], f16)
        nc.gpsimd.dma_start(out=vT_sb,
                            in_=Vw.rearrange("(k p) one -> p (k one)", p=128))
        qbT_sb = singles.tile([128, KU], f32)
        nc.sync.dma_start(out=qbT_sb, in_=qb.rearrange("(k p) -> p k", p=128))

        # ---- q_projT = W1.T @ query.T + qbT : [u-part, KU, BPC] (fp32) ----
        q_sb = singles.tile([BPC, D], f32)
        nc.sync.dma_start(out=q_sb, in_=query)
        qT = singles.tile([128, KD, BPC], f32)
        for kd in range(KD):
            tp = tp_psum.tile([128, BPC], f32, tag="tp")
            nc.tensor.transpose(tp, q_sb[:, kd * 128:(kd + 1) * 128],
                                ident_f[0:BPC, 0:BPC])
            nc.scalar.copy(out=qT[:, kd, :], in_=tp)
        qpT = singles.tile([128, KU, BPC], f32)
        for ku in range(KU):
            qp_ps = vp_psum.tile([128, BPC], f32, tag="vp")
            for kd in range(KD):
                nc.tensor.matmul(qp_ps, lhsT=w1_sb[:, kd, ku * 128:(ku + 1) * 128],
                                 rhs=qT[:, kd, :], start=(kd == 0), stop=(kd == KD - 1))
            nc.vector.tensor_scalar_add(qpT[:, ku, :], qp_ps, qbT_sb[:, ku:ku + 1])

        # ---- main loop over batch elements ----
        for bi, b in enumerate(batch_list):
            if bi == 0:
                # batch 0 via HWDGE fp32 (SWDGE needs ~12us to spin up);
                # cast to fp16 on-chip for the context matmul
                vsrc = values[b].rearrange("(p n) d -> p n d", p=128)
                stage = stage_pool.tile([128, NT, D], f32, tag="stage0")
                for q in range(4):
                    nc.sync.dma_start(out=stage[:, q * 4:(q + 1) * 4, :],
                                      in_=vsrc[:, q * 4:(q + 1) * 4, :])
                vq = []
                for q in range(2):
                    t = vnat_pool.tile([128, NQ, D], f16, tag=f"vnat{q}")
                    h = stage[:, q * NQ:(q + 1) * NQ, :]
                    nc.vector.tensor_copy(out=t[:, 0:NQ // 2, :],
                                          in_=h[:, 0:NQ // 2, :])
                    nc.scalar.copy(out=t[:, NQ // 2:, :], in_=h[:, NQ // 2:, :])
                    vq.append(t)
            else:
                vq = load_vnat(b)

            def vnat(n):
                return vq[n // NQ][:, n % NQ, :]

            def vnat_src(n):
                # transpose source: fp32 staging for batch 0, fp16 otherwise
                if bi == 0:
                    return stage[:, n, :], ident_f
                return vq[n // NQ][:, n % NQ, :], ident_h

            # transpose to vt[d-part, kd, j], j = n*128 + pcol  (t = pcol*16 + n)
            vt = vt_pool.tile([128, KD, T], f16, tag="vt")
            for n in range(NT):
                tsrc, tident = vnat_src(n)
                tp = tp_psum.tile([128, KD * 128], tsrc.dtype, tag="tp")
                for kd in range(KD):
                    nc.tensor.transpose(
                        tp[:, kd * 128:(kd + 1) * 128],
                        tsrc[:, kd * 128:(kd + 1) * 128],
                        tident,
                    )
                nc.vector.tensor_copy(
                    out=vt[:, :, n * 128:(n + 1) * 128],
                    in_=tp.rearrange("p (k t) -> p k t", k=KD))

            # v_projT (W2 stationary) + tanh(+bias) + score MMs
            score_row = small.tile([1, T], f32, tag="score_row")
            for tchunk in range(TC):
                jsl = slice(tchunk * 512, (tchunk + 1) * 512)
                sc_ps = sc_psum.tile([1, 512], f32, tag="sc")
                for ku in range(KU):
                    vp = vp_psum.tile([128, 512], f32, tag="vp")
                    for kd in range(KD):
                        nc.tensor.matmul(
                            vp,
                            lhsT=w2_sb[:, kd, ku * 128:(ku + 1) * 128],
                            rhs=vt[:, kd, jsl],
                            start=(kd == 0), stop=(kd == KD - 1),
                        )
                    z = z_pool.tile([128, 512], f16, tag="z")
                    nc.scalar.activation(out=z, in_=vp,
                                         func=mybir.ActivationFunctionType.Tanh,
                                         bias=qpT[:, ku, b:b + 1], scale=1.0)
                    nc.tensor.matmul(sc_ps, lhsT=vT_sb[:, ku:ku + 1], rhs=z,
                                     start=(ku == 0), stop=(ku == KU - 1))
                nc.scalar.copy(out=score_row[:, jsl], in_=sc_ps)

            # scatter score row (j = n*128 + p) -> [128, 16] columns via DRAM
            nc.sync.dma_start(out=score_dram[b][None, :], in_=score_row)
            score_sb = small.tile([128, NT], f32, tag="score")
            nc.sync.dma_start(out=score_sb,
                              in_=score_dram[b].rearrange("(n p) -> p n", p=128))

            # softmax over all T in fp32 (no max-subtraction; scores bounded)
            exp_sb = small.tile([128, NT], f32, tag="exp")
            nc.scalar.activation(out=exp_sb, in_=score_sb,
                                 func=mybir.ActivationFunctionType.Exp)
            exp_h = small.tile([128, NT], f16, tag="exph")
            nc.vector.tensor_copy(out=exp_h, in_=exp_sb)
            sum_col = small.tile([128, 1], f32, tag="sumcol")
            nc.vector.reduce_sum(sum_col, exp_sb, axis=mybir.AxisListType.X)
            tot_ps = sc_psum.tile([1, 1], f32, tag="sc")
            nc.tensor.matmul(tot_ps, lhsT=sum_col, rhs=ones_col,
                             start=True, stop=True)
            tot_sb = small.tile([1, 1], f32, tag="tot_sb")
            nc.scalar.copy(out=tot_sb, in_=tot_ps)
            rb_ps = sc_psum.tile([128, 1], f32, tag="sc")
            nc.tensor.matmul(rb_ps, lhsT=ones_row, rhs=tot_sb,
                             start=True, stop=True)
            recip_sb = small.tile([128, 1], f32, tag="recip")
            nc.vector.reciprocal(recip_sb, rb_ps)

            # attn output: attn[p*16+n] = exp[p, n] * recip -> contiguous store
            attn_sb = small.tile([128, NT], f32, tag="attn")
            nc.vector.tensor_scalar_mul(attn_sb, exp_sb, recip_sb)
            nc.sync.dma_start(out=attn_out[b].rearrange("(p n) -> p n", p=128),
                              in_=attn_sb)

            # context = (exp . values) / tot   (fp16 operands, fp32 accum)
            ctx_ps = sc_psum.tile([1, D], f32, tag="sc")
            for n in range(NT):
                nc.tensor.matmul(ctx_ps, lhsT=exp_h[:, n:n + 1],
                                 rhs=vnat(n),
                                 start=(n == 0), stop=(n == NT - 1))
            ctx_sb = small.tile([1, D], f32, tag="ctx_sb")
            nc.scalar.activation(out=ctx_sb, in_=ctx_ps,
                                 func=mybir.ActivationFunctionType.Copy,
                                 bias=0.0, scale=recip_sb[0:1, 0:1])
            nc.sync.dma_start(out=ctx_out[b][None, :], in_=ctx_sb)

    nc.compile()
    return nc


def _get_program():
    if "nc" not in _CACHE:
        _CACHE["nc"] = _build_program()
    return _CACHE["nc"]


def _make_in_maps(query, values, W1_w, W1_b, W2_w, W2_b, V_w, V_b):
    query = np.ascontiguousarray(np.asarray(query, dtype=np.float32))
    values = np.ascontiguousarray(np.asarray(values, dtype=np.float32))
    W1_w = np.ascontiguousarray(np.asarray(W1_w, dtype=np.float32))
    W2_w = np.ascontiguousarray(np.asarray(W2_w, dtype=np.float32))
    V_w = np.ascontiguousarray(np.asarray(V_w, dtype=np.float32))
    qb = (np.asarray(W1_b, dtype=np.float32)
          + np.asarray(W2_b, dtype=np.float32)).astype(np.float32)
    # V_b drops out of softmax (shift invariance) -> not an input to the kernel
    in_maps = []
    for c in range(N_CORES):
        sl = slice(c * BPC, (c + 1) * BPC)
        in_maps.append({
            "query": np.ascontiguousarray(query[sl]),
            "values": np.ascontiguousarray(values[sl]),
            "W1_w": W1_w,
            "W2_w": W2_w,
            "V_w": V_w,
            "qb": qb,
        })
    return in_maps


def kernel(query, values, W1_w, W1_b, W2_w, W2_b, V_w, V_b):
    from concourse.bass_utils import run_bass_kernel_spmd

    nc = _get_program()
    in_maps = _make_in_maps(query, values, W1_w, W1_b, W2_w, W2_b, V_w, V_b)
    res = run_bass_kernel_spmd(nc, in_maps, list(range(N_CORES)))
    context = np.concatenate([res.results[c]["context"] for c in range(N_CORES)], axis=0)
    attn = np.concatenate([res.results[c]["attn"] for c in range(N_CORES)], axis=0)
    return context.astype(np.float32), attn[..., None].astype(np.float32)
